# revision 36
# baseline (speedup 1.0000x reference)
"""Bar-level attention Trainium2 kernel (8 NeuronCores, head-parallel).

Contract: kernel(**inputs) takes the FULL inputs from setup_inputs() and
returns the FULL [1, 2048, 512] float32 output.

Strategy (one head per core, 8 heads / 8 cores), all matmul IO in bf16
(PSUM accumulation stays fp32):
  - Host: XT [512, 2048] bf16; per-head packs:
      wqk [128, 4*128]: per 128-row contraction chunk kc, cols 0:64 =
        (Wq_h.T * scale)[kc], cols 64:128 = Wk_h.T[kc]  -> Q^T and K^T come
        out of ONE matmul stream (stacked stationary, 128 out rows).
      wv  [128, 4*64]: Wv_h.T chunks (V computed in [key, dh] layout with
        64-wide moving operand).
      wot2 [128, 512]: rows 0:64 = g*Wo_h.T, rows 64:128 = (1-g)*Wo_h.T
        (gate folded into the output projection).
      maskp: per-key-chunk bar-equality bands, packed to their true widths.
  - Device per core:
      warmup dummy matmuls (PE p-state ramp), projections pipelined under
      the XT DMA (per-contraction-chunk accumulation passes), then per
      query half: scores S^T = K_c^T Q (keys on partitions), Exp on Act
      engine (the critical resource: ~33us of column time), global AV and
      masked local AV accumulate in PSUM with a trailing ones column giving
      softmax denominators for free.  PSUM has_written semantics (start=True
      clears the whole bank; cleared words are overwritten, not
      accumulated) let local AV pieces accumulate without zero-init.
      Transition: Pool broadcasts the denominator rows, DVE divides the AV
      rows and stacks local (rows 0:64) over global (rows 64:128) in bf16;
      output projection is then ONE matmul per 128-query chunk against
      wot2, drained round-robin over Act/DVE/Pool into bf16 and DMA'd out.
  - Host: sum the 8 bf16 partials in fp32 (contraction-sharded Wo) + bo.

The global-attention additive bias in the reference is per-query and
softmax is shift-invariant per row, so it drops out exactly.
"""

import numpy as np

S = 2048
D = 512
H = 8
DH = 64
SCALE = 1.0 / np.sqrt(DH)
NCHUNK = S // 128       # 16 key chunks of 128
NHALF = 2               # query halves of 1024
QHALF = S // NHALF
VSTRIDE = 66            # per-chunk stride in the packed V tile (64 + ones + pad)


def _legalize_waits(nc, mybir):
    """This walrus codegen accepts at most ONE sync wait per instruction.
    Split any instruction carrying N>1 waits into N-1 preceding single-wait
    NoOps on the same engine (waits execute in order on the sequencer)."""
    ctr = 0
    for f in nc.m.functions:
        for b in f.blocks:
            insts = b.instructions
            if not any(i.sync_info and len(i.sync_info.on_wait) > 1 for i in insts):
                continue
            new = []
            for ins in insts:
                si = ins.sync_info
                if si is not None and len(si.on_wait) > 1:
                    waits = list(si.on_wait)
                    for w in waits[:-1]:
                        ctr += 1
                        nop = mybir.InstNoOp(name=f"waitsplit-{ctr}", engine=ins.engine)
                        nop.sync_info = mybir.SyncInfo(on_wait=[w], on_update=[])
                        new.append(nop)
                    ins.sync_info = mybir.SyncInfo(
                        on_wait=[waits[-1]], on_update=list(si.on_update)
                    )
                new.append(ins)
            insts.clear()
            insts.extend(new)
    return ctr


def _bar_bounds(bp):
    """bp: sorted int array [S] -> list of (start, end) per bar."""
    change = np.nonzero(np.diff(bp))[0] + 1
    starts = np.concatenate([[0], change])
    ends = np.concatenate([change, [len(bp)]])
    return list(zip(starts.tolist(), ends.tolist()))


def _attn_layout(bars):
    """Static layout derived from the (baked) bar boundaries.

    band[c]  = (blo, bhi): union query span of bars intersecting key chunk c
    moff[c]  = column offset of chunk c's band in the packed mask tile
    segs[(hq,c)] = (hs, he) band clipped to the query half, or None
    splits[(hq,c)] = [(a, b, start, stop)]: seg split at 512-col PSUM bank
      boundaries; start/stop mark the first/last matmul touching each bank
      of the local-AV accumulator (has_written bank epoch management).
    """
    band = []
    for c in range(NCHUNK):
        klo, khi = c * 128, (c + 1) * 128
        bs = [b for b in bars if b[1] > klo and b[0] < khi]
        blo, bhi = bs[0][0], bs[-1][1]
        if bhi - blo > 512:
            return None
        band.append((blo, bhi))
    widths = [(b[1] - b[0] + 1) // 2 * 2 for b in band]  # pad even
    moff = [0] * NCHUNK
    for c in range(1, NCHUNK):
        moff[c] = moff[c - 1] + widths[c - 1]
    mw = moff[-1] + widths[-1]

    segs = {}
    splits = {}
    for hq in range(NHALF):
        qlo, qhi = hq * QHALF, (hq + 1) * QHALF
        bank_touch = {}
        for c in range(NCHUNK):
            blo, bhi = band[c]
            hs, he = max(blo, qlo), min(bhi, qhi)
            if hs >= he:
                segs[(hq, c)] = None
                continue
            segs[(hq, c)] = (hs, he)
            ss = []
            a = hs
            while a < he:
                b = min(he, qlo + ((a - qlo) // 512 + 1) * 512)
                bank_touch.setdefault((a - qlo) // 512, []).append((c, len(ss)))
                ss.append([a, b, False, False])
                a = b
            splits[(hq, c)] = ss
        for _, lst in bank_touch.items():
            c0, i0 = lst[0]
            splits[(hq, c0)][i0][2] = True
            c1, i1 = lst[-1]
            splits[(hq, c1)][i1][3] = True
    return band, moff, mw, segs, splits


def _build(bars):
    import concourse.bass as bass
    import concourse.tile as tile
    import concourse.mybir as mybir

    dt = mybir.dt
    AF = mybir.ActivationFunctionType
    OP = mybir.AluOpType
    f32 = dt.float32
    f32r = dt.float32r
    bf16 = dt.bfloat16

    lay = _attn_layout(bars)
    assert lay is not None
    band, moff, mw, segs, splits = lay

    nc = bass.Bass()
    xt_d = nc.dram_tensor("xt", [D, S], bf16, kind="ExternalInput")
    wqk_d = nc.dram_tensor("wqk", [128, 4 * 128], bf16, kind="ExternalInput")
    wv_d = nc.dram_tensor("wv", [128, 4 * 64], bf16, kind="ExternalInput")
    wot_d = nc.dram_tensor("wot", [DH, D], f32r, kind="ExternalInput")
    maskp_d = nc.dram_tensor("maskp", [128, mw], bf16, kind="ExternalInput")
    smalls_d = nc.dram_tensor("smalls", [128, 4], f32, kind="ExternalInput")
    out_d = nc.dram_tensor("out_partial", [S, D], bf16, kind="ExternalOutput")

    with tile.TileContext(nc, pool_alloc_mode="queue") as tc:
        with tc.tile_pool(name="persist", bufs=1) as p_keep:
            qt = p_keep.tile([DH, S], bf16, tag="qt")
            kt = p_keep.tile([DH, S], bf16, tag="kt")
            vt = p_keep.tile([128, NCHUNK * VSTRIDE], bf16, tag="vt")
            wot = p_keep.tile([DH, D], f32r, tag="wot")
            maskp = p_keep.tile([128, mw], bf16, tag="maskp")
            # smalls [128,4] f32: rows 0:64 col0 = bq*scale, col1 = bk;
            # all rows: col2 = sigmoid(gate), col3 = 1-sigmoid(gate)
            smalls = p_keep.tile([128, 4], f32, tag="smalls")
            obuf = [
                p_keep.tile([128, (4 if i < 3 else 2) * D], bf16, tag=f"ob{i}",
                            name=f"obuf{i}")
                for i in range(5)
            ]
            wtiny = p_keep.tile([128, 128], bf16, tag="wtiny")
            # transposed denominators / reciprocals: cols hq*16+jj = local,
            # hq*16+8+jj = global; r2[p, hq*16+jj] = gate/l_local(q) for
            # q = hq*1024 + p*8 + jj (the stride-8 interleave makes each
            # output chunk's scales one column)
            l2 = p_keep.tile([128, 32], f32, tag="l2")
            r2 = p_keep.tile([128, 32], f32, tag="r2")
            ol_sb = [
                p_keep.tile([DH + 1, QHALF], f32r, tag=f"olsb{h}", name=f"ol_sb{h}")
                for h in range(NHALF)
            ]
            og_sb = [
                p_keep.tile([DH + 1, QHALF], f32r, tag=f"ogsb{h}", name=f"og_sb{h}")
                for h in range(NHALF)
            ]

            # ---- PE p-state warmup: keep PE busy from t~0 so the 3us ramp
            # to max clock completes under the input DMA.  The dummies read
            # uninitialized SBUF on purpose: their PSUM output is never
            # consumed, and waiting on a memset would delay the ramp.
            # ones columns of the packed V tile (col 64 of each 66-wide chunk)
            nc.gpsimd.memset(
                vt.rearrange("p (c j) -> p c j", j=VSTRIDE)[:, :, DH : DH + 1], 1.0
            )
            nc.gpsimd.memset(wtiny[0:1, 0:1], 0.0)  # force allocation only
            # warmup holds 4 banks so the score pool inherits banks with
            # no dependency on the projection drains
            with tc.tile_pool(name="pwarm", bufs=1, space="PSUM") as p_w:
                wpa = p_w.tile([128, QHALF], f32, tag="wpa")
                wpb = p_w.tile([128, QHALF], f32, tag="wpb")
                for i in range(28):
                    wp = wpa if i % 2 == 0 else wpb
                    nc.tensor.matmul(
                        wp[:, 0:128], wtiny[:], wtiny[:],
                        start=True, stop=True, skip_group_check=True,
                    )

            # ---------------- projections + attention ----------------
            # Software-pipelined: half-0 projections run under the xt DMA;
            # half-1 projection passes + drains are injected into half-0's
            # score/exp stream (Act only ever runs exp once the loop
            # starts); og/lAV accumulation for early chunks is emitted late
            # so its PSUM banks (freed by the half-1 projection pool) are
            # ready without stalling the in-order engines.
            with tc.tile_pool(name="inp", bufs=1) as p_in:
                wqk = p_in.tile([128, 4 * 128], bf16, tag="wqk")
                wv = p_in.tile([128, 4 * 64], bf16, tag="wv")
                xts = [
                    p_in.tile([128, S], bf16, tag=f"xt{i}", name=f"xts{i}")
                    for i in range(4)
                ]
                # DMA issue order == service order: first-needed first.
                nc.sync.dma_start(xts[0][:, 0:QHALF], xt_d[0:128, 0:QHALF])
                nc.sync.dma_start(wqk[:], wqk_d[:])
                nc.sync.dma_start(smalls[:], smalls_d[:])
                nc.sync.dma_start(xts[1][:, 0:QHALF], xt_d[128:256, 0:QHALF])
                nc.sync.dma_start(xts[2][:, 0:QHALF], xt_d[256:384, 0:QHALF])
                nc.sync.dma_start(xts[3][:, 0:QHALF], xt_d[384:512, 0:QHALF])
                nc.sync.dma_start(wv[:], wv_d[:])
                nc.sync.dma_start(maskp[:], maskp_d[:])
                for kc in range(4):
                    nc.sync.dma_start(
                        xts[kc][:, QHALF:S],
                        xt_d[kc * 128 : (kc + 1) * 128, QHALF:S],
                    )
                nc.sync.dma_start(wot[:], wot_d[:])

                def qk_pass(qkp, h, kc):
                    hq0 = h * QHALF
                    for n in range(QHALF // 512):
                        nc.tensor.matmul(
                            qkp[:, n * 512 : (n + 1) * 512],
                            wqk[:, kc * 128 : (kc + 1) * 128],
                            xts[kc][:, hq0 + n * 512 : hq0 + (n + 1) * 512],
                            start=(kc == 0),
                            stop=(kc == 3),
                        )

                def v_pass(vp, h, kc):
                    hq0 = h * QHALF
                    for cc in range(8):
                        nc.tensor.matmul(
                            vp[:, cc * DH : (cc + 1) * DH],
                            xts[kc][:, hq0 + cc * 128 : hq0 + (cc + 1) * 128],
                            wv[:, kc * DH : (kc + 1) * DH],
                            start=(kc == 0 and cc == 0),
                            stop=(kc == 3),
                            skip_group_check=True,
                        )

                def v_drain(vp, h, act=False):
                    dstv = vt[
                        :, h * 8 * VSTRIDE : (h + 1) * 8 * VSTRIDE
                    ].rearrange("p (c j) -> p c j", j=VSTRIDE)[:, :, 0:DH]
                    src_ = vp[:].rearrange("p (c j) -> p c j", j=DH)
                    if act:
                        nc.scalar.copy(dstv, src_)
                    else:
                        nc.vector.tensor_copy(dstv, src_)

                with (
                    tc.tile_pool(name="ps", bufs=2, space="PSUM") as p_s,
                    tc.tile_pool(name="pe", bufs=12) as p_e,
                    tc.tile_pool(name="pel", bufs=12) as p_el,
                ):
                    es = {}
                    els = {}

                    def emit_sc_exp(hq, c):
                        qlo = hq * QHALF
                        sc = p_s.tile([128, QHALF], f32, tag="s", name="sc")
                        for n in range(QHALF // 512):
                            nc.tensor.matmul(
                                sc[:, n * 512 : (n + 1) * 512],
                                kt[:, c * 128 : (c + 1) * 128],
                                qt[:, qlo + n * 512 : qlo + (n + 1) * 512],
                                start=True,
                                stop=True,
                            )
                        ec = p_e.tile([128, QHALF], bf16, tag="e", name="ec")
                        nc.scalar.activation(ec[:], sc[:], AF.Exp)
                        es[(hq, c)] = ec
                        seg = segs[(hq, c)]
                        if seg is not None:
                            hs, he = seg
                            blo = band[c][0]
                            el = p_el.tile([128, 512], bf16, tag="el", name="el")
                            nc.vector.tensor_mul(
                                el[:, 0 : he - hs],
                                ec[:, hs - qlo : he - qlo],
                                maskp[:, moff[c] + hs - blo : moff[c] + he - blo],
                            )
                            els[(hq, c)] = el

                    def emit_og_lav(hq, c, og, ol):
                        qlo = hq * QHALF
                        ec = es.pop((hq, c))
                        vst = vt[:, c * VSTRIDE : c * VSTRIDE + DH + 1]
                        for n in range(QHALF // 512):
                            nc.tensor.matmul(
                                og[:, n * 512 : (n + 1) * 512],
                                vst,
                                ec[:, n * 512 : (n + 1) * 512],
                                start=(c == 0),
                                stop=(c == NCHUNK - 1),
                            )
                        if (hq, c) in els:
                            el = els.pop((hq, c))
                            hs = segs[(hq, c)][0]
                            for (a, b, st, sp) in splits[(hq, c)]:
                                nc.tensor.matmul(
                                    ol[:, a - qlo : b - qlo],
                                    vst,
                                    el[:, a - hs : b - hs],
                                    start=st,
                                    stop=sp,
                                    skip_group_check=True,
                                )

                    def transition(hq, og, ol):
                        # final half: ol on Act (it feeds the t1 chain and
                        # Act is idle after the last exp), og on DVE ahead
                        # of the stt stream; mid-loop half: both on DVE to
                        # keep Act exp-only
                        if hq == NHALF - 1:
                            nc.scalar.copy(ol_sb[hq][:], ol[:])
                            nc.vector.tensor_copy(og_sb[hq][:], og[:])
                        else:
                            nc.vector.tensor_copy(ol_sb[hq][:], ol[:])
                            nc.vector.tensor_copy(og_sb[hq][:], og[:])
                        c0 = hq * 16
                        nc.sync.dma_start(
                            l2[:, c0 : c0 + 8],
                            ol_sb[hq][DH : DH + 1, :].bitcast(f32),
                        )
                        nc.sync.dma_start(
                            l2[:, c0 + 8 : c0 + 16],
                            og_sb[hq][DH : DH + 1, :].bitcast(f32),
                        )
                        nc.vector.reciprocal(
                            r2[:, c0 : c0 + 16], l2[:, c0 : c0 + 16]
                        )
                        nc.vector.tensor_scalar_mul(
                            r2[:, c0 : c0 + 8], r2[:, c0 : c0 + 8],
                            smalls[:, 2:3],
                        )
                        nc.vector.tensor_scalar_mul(
                            r2[:, c0 + 8 : c0 + 16], r2[:, c0 + 8 : c0 + 16],
                            smalls[:, 3:4],
                        )

                    with tc.tile_pool(name="ph0", bufs=1, space="PSUM") as ph0:
                        qk0 = ph0.tile([128, QHALF], f32, tag="qk0")
                        v0 = ph0.tile([128, 8 * DH], f32, tag="v0")
                        for kc in range(4):
                            qk_pass(qk0, 0, kc)
                        nc.scalar.copy(qt[:, 0:QHALF], qk0[0:DH, :])
                        nc.scalar.copy(kt[:, 0:256], qk0[DH:128, 0:256])
                        nc.vector.tensor_copy(
                            kt[:, 256:QHALF], qk0[DH:128, 256:QHALF]
                        )
                        emit_sc_exp(0, 0)
                        for kc in range(4):
                            v_pass(v0, 0, kc)
                        v_drain(v0, 0)
                        emit_sc_exp(0, 1)

                    with tc.tile_pool(name="ph1", bufs=1, space="PSUM") as ph1:
                        qk1 = ph1.tile([128, QHALF], f32, tag="qk1")
                        v1 = ph1.tile([128, 8 * DH], f32, tag="v1")
                        emit_sc_exp(0, 2)
                        for c in range(3, 7):
                            emit_sc_exp(0, c)
                            qk_pass(qk1, 1, c - 3)
                        emit_sc_exp(0, 7)
                        # kt for chunks 8..9 first so the half-0 score stream
                        # never starves, then V, then qt (only needed for the
                        # half-1 scores much later)
                        nc.vector.tensor_copy(
                            kt[:, QHALF : QHALF + 256], qk1[DH:128, 0:256]
                        )
                        nc.vector.tensor_copy(
                            kt[:, QHALF + 256 : S], qk1[DH:128, 256:QHALF]
                        )
                        for kc in range(4):
                            v_pass(v1, 1, kc)
                        v_drain(v1, 1)
                        nc.vector.tensor_copy(qt[:, QHALF:S], qk1[0:DH, :])

                    with (
                        tc.tile_pool(name="pog", bufs=1, space="PSUM") as p_og,
                        tc.tile_pool(name="pol", bufs=1, space="PSUM") as p_ol,
                    ):
                        og0 = p_og.tile([DH + 1, QHALF], f32, tag="og", name="og0")
                        ol0 = p_ol.tile([DH + 1, QHALF], f32, tag="ol", name="ol0")
                        for c in range(8):
                            emit_sc_exp(0, 8 + c)
                            emit_og_lav(0, c, og0, ol0)
                        for c in range(8, NCHUNK - 2):
                            emit_og_lav(0, c, og0, ol0)
                        emit_sc_exp(1, 0)
                        emit_og_lav(0, NCHUNK - 2, og0, ol0)
                        emit_sc_exp(1, 1)
                        emit_og_lav(0, NCHUNK - 1, og0, ol0)
                        transition(0, og0, ol0)

                        og1 = p_og.tile([DH + 1, QHALF], f32, tag="og", name="og1")
                        ol1 = p_ol.tile([DH + 1, QHALF], f32, tag="ol", name="ol1")
                        for c in range(2, NCHUNK - 1):
                            emit_sc_exp(1, c)
                            emit_og_lav(1, c - 2, og1, ol1)
                        emit_sc_exp(1, NCHUNK - 1)
                        for c in range(NCHUNK - 3, NCHUNK):
                            emit_og_lav(1, c, og1, ol1)
                        transition(1, og1, ol1)

            # ---------------- output projection ----------------
            # interleaved query chunks (queries jj, jj+8, ...): partition p
            # of chunk jj is query hq*1024 + p*8 + jj, so r2 column
            # hq*16+jj is exactly the per-partition scale vector.  Per
            # chunk: local+global projections into one 2-bank psum pair,
            # t1 = lp*r_l on Act (activation scale), out = gp*r_g + t1 on
            # DVE (scalar_tensor_tensor), both reading PSUM directly.
            with (
                tc.tile_pool(name="pout", bufs=4, space="PSUM") as p_o,
                tc.tile_pool(name="pt1", bufs=4) as p_t1,
            ):
                for hq in range(NHALF):
                    c0 = hq * 16
                    for jj in range(8):
                        op = p_o.tile([128, 2 * D], f32, tag="op")
                        nc.tensor.matmul(
                            op[:, 0:D],
                            ol_sb[hq][0:DH, jj:QHALF:8],
                            wot[:],
                            start=True,
                            stop=True,
                        )
                        nc.tensor.matmul(
                            op[:, D : 2 * D],
                            og_sb[hq][0:DH, jj:QHALF:8],
                            wot[:],
                            start=True,
                            stop=True,
                        )
                        t1 = p_t1.tile([128, D], bf16, tag="t1")
                        nc.scalar.activation(
                            t1[:], op[:, 0:D], AF.Identity,
                            scale=r2[:, c0 + jj : c0 + jj + 1],
                        )
                        ci = hq * 8 + jj
                        gi = ci // 4 if ci < 12 else 3 + (ci - 12) // 2
                        g0 = gi * 4 if gi < 3 else 12 + (gi - 3) * 2
                        gw = 4 if gi < 3 else 2
                        grp = obuf[gi]
                        nc.vector.scalar_tensor_tensor(
                            grp[:, (ci - g0) * D : (ci - g0 + 1) * D],
                            op[:, D : 2 * D],
                            r2[:, c0 + 8 + jj : c0 + 9 + jj],
                            t1[:],
                            OP.mult,
                            OP.add,
                        )
                        if ci == g0 + gw - 1:
                            dst = out_d[
                                hq * QHALF : (hq + 1) * QHALF, :
                            ].rearrange("(p j) c -> p j c", j=8)[
                                :, jj - gw + 1 : jj + 1, :
                            ]
                            nc.sync.dma_start(
                                dst, grp[:].rearrange("p (j c) -> p j c", j=gw)
                            )

    _legalize_waits(nc, mybir)
    return nc


_CACHE = {}


def _get_built(bar_key, bars):
    if bar_key not in _CACHE:
        _CACHE[bar_key] = _build(bars)
    return _CACHE[bar_key]


def _np_reference(hidden_states, bar_positions, attention_mask, Wq, bq, Wk, bk,
                  Wv, bv, Wo, bo, bar_emb, gate):
    """Plain numpy fallback (only used if inputs violate baked assumptions)."""
    B, S_, _ = hidden_states.shape
    x = hidden_states.astype(np.float64)
    q = (x @ Wq.T + bq).reshape(B, S_, H, DH).transpose(0, 2, 1, 3)
    k = (x @ Wk.T + bk).reshape(B, S_, H, DH).transpose(0, 2, 1, 3)
    v = (x @ Wv.T + bv).reshape(B, S_, H, DH).transpose(0, 2, 1, 3)
    scores = np.einsum("bhqd,bhkd->bhqk", q, k) * SCALE
    pad = attention_mask[:, None, None, :]
    bar_mask = (bar_positions[:, :, None] == bar_positions[:, None, :])[:, None]
    NEG = -np.inf

    def softmax(s):
        s = s - s.max(-1, keepdims=True)
        e = np.exp(s)
        return e / e.sum(-1, keepdims=True)

    local = softmax(np.where(bar_mask & pad, scores, NEG))
    emb = bar_emb[np.asarray(bar_positions) % bar_emb.shape[0]]
    bias = np.sum(emb * emb, axis=-1)
    glob = softmax(np.where(pad, scores + bias[:, None, :, None], NEG))
    la = np.einsum("bhqk,bhkd->bhqd", local, v)
    ga = np.einsum("bhqk,bhkd->bhqd", glob, v)
    g = 1.0 / (1.0 + np.exp(-gate))[None, :, None, None]
    comb = g * la + (1.0 - g) * ga
    out = comb.transpose(0, 2, 1, 3).reshape(B, S_, H * DH)
    return (out @ Wo.T + bo).astype(np.float32)


def kernel(**inputs):
    import ml_dtypes

    bf16 = ml_dtypes.bfloat16

    hidden_states = np.asarray(inputs["hidden_states"], dtype=np.float32)
    bar_positions = np.asarray(inputs["bar_positions"])
    attention_mask = np.asarray(inputs["attention_mask"])
    Wq = np.asarray(inputs["Wq"], dtype=np.float32)
    bq = np.asarray(inputs["bq"], dtype=np.float32)
    Wk = np.asarray(inputs["Wk"], dtype=np.float32)
    bk = np.asarray(inputs["bk"], dtype=np.float32)
    Wv = np.asarray(inputs["Wv"], dtype=np.float32)
    bv = np.asarray(inputs["bv"], dtype=np.float32)
    Wo = np.asarray(inputs["Wo"], dtype=np.float32)
    bo = np.asarray(inputs["bo"], dtype=np.float32)
    gate = np.asarray(inputs["gate"], dtype=np.float32)

    bp = bar_positions[0].astype(np.int64)
    bars = _bar_bounds(bp)
    if (
        hidden_states.shape != (1, S, D)
        or not bool(attention_mask.all())
        or not bool((np.diff(bp) >= 0).all())
        or bool(np.abs(bv).max() > 0)
        or bool(np.abs(bq).max() > 0)
        or bool(np.abs(bk).max() > 0)
        or _attn_layout(bars) is None
    ):
        return _np_reference(
            hidden_states, bar_positions, attention_mask, Wq, bq, Wk, bk,
            Wv, bv, Wo, bo, np.asarray(inputs["bar_emb"], dtype=np.float32), gate,
        )

    nc = _get_built(bp.tobytes(), bars)
    band, moff, mw, _, _ = _attn_layout(bars)

    # packed mask bands (same for every core)
    maskp = np.zeros((128, mw), dtype=bf16)
    for c in range(NCHUNK):
        klo, khi = c * 128, (c + 1) * 128
        blo, bhi = band[c]
        eq = bp[klo:khi, None] == bp[None, blo:bhi]
        maskp[:, moff[c] : moff[c] + (bhi - blo)] = eq.astype(bf16)

    xt = np.ascontiguousarray(hidden_states[0].T).astype(bf16)  # [512, 2048]
    g = 1.0 / (1.0 + np.exp(-gate.astype(np.float64)))  # sigmoid, [H]
    in_maps = []
    for h in range(H):
        sl = slice(h * DH, (h + 1) * DH)
        wqt = Wq[sl, :].T * np.float32(SCALE)  # [512, 64]
        wkt = Wk[sl, :].T
        wvt = Wv[sl, :].T
        wqk = np.empty((128, 4 * 128), dtype=np.float32)
        wv = np.empty((128, 4 * 64), dtype=np.float32)
        for kc in range(4):
            r = slice(kc * 128, (kc + 1) * 128)
            wqk[:, kc * 128 : kc * 128 + 64] = wqt[r]
            wqk[:, kc * 128 + 64 : (kc + 1) * 128] = wkt[r]
            wv[:, kc * 64 : (kc + 1) * 64] = wvt[r]
        wot = np.ascontiguousarray(Wo[:, sl].T)  # [64, 512] fp32
        smalls = np.zeros((128, 4), dtype=np.float32)
        smalls[0:DH, 0] = bq[sl] * np.float32(SCALE)
        smalls[0:DH, 1] = bk[sl]
        smalls[:, 2] = np.float32(g[h])
        smalls[:, 3] = np.float32(1.0 - g[h])
        in_maps.append(
            {"xt": xt, "wqk": wqk.astype(bf16), "wv": wv.astype(bf16),
             "wot": wot, "maskp": maskp, "smalls": smalls}
        )

    res = _run_spmd(nc, in_maps)
    out = np.zeros((S, D), dtype=np.float32)
    for h in range(H):
        out += np.asarray(res.results[h]["out_partial"], dtype=np.float32)
    out += bo
    return out.reshape(1, S, D)


def _run_spmd(nc, in_maps, **kw):
    from concourse.bass_utils import run_bass_kernel_spmd

    return run_bass_kernel_spmd(nc, in_maps, list(range(H)), **kw)
